# revision 14
# baseline (speedup 1.0000x reference)
"""Multi-head self-attention (B=2, S=2048, D=1024, H=16) on 8 TRN2 NeuronCores.

Sharding: batch x head-group parallel. Core c handles batch c//4 and the
4 heads 4*(c%4)..4*(c%4)+3. Each core reads only its batch's tokens
(halves x/y DMA vs pure head-parallel), computes Q/K/V for its 4 heads,
full non-causal softmax attention for its 4 (head) units, and the partial
output projection y_c = sum_h out_h @ wo[h]. Host sums the 4 partials per
batch. Host pre-transposes x to xT and pre-rounds x and all weights to
tf32 bit patterns so the device DMAs straight into float32r tiles.

Device dataflow per core (heads grouped in 2 pairs of 2; head-dim on
partitions; pair p owns partitions [0:64] for head 2p, [64:128] for 2p+1):
  k2t[p] / q2tz[h] / out2t[p] [128, T]; x resident in SBUF [128, ndc, T]
  proj: psum = sum_dc w[:,dc,p*128:+128]^T @ x[:,dc,chunk]   (PSUM accum)
  v --PE transpose--> vnat[h] [k, 64] bf16 (+ ones column -> denominator)
  scoresT[k, q] = K Q^T with per-head zero-padded Q (K=128 contraction);
    score matmuls write PAIRS of PSUM banks, one ACT exp instruction
    covers both ([128, 1024]) to amortize ACT fixed overheads
  poT[65, q] += vnat[h,kt]^T @ exp  accumulated over kt (PSUM)
  normalize: dr=[1,512] raw denominator -> reciprocal_approx_fast ->
    K=1 matmul broadcast -> out2t[hp, q] = po[0:64] * rcp (DVE)
  y[s, :] = sum_p out2t[p][:, s-tile]^T @ wo[p]  -> SBUF -> DMA

Attention operands (k2t, q2tz, vnat, exp) are bf16 (same 1 cycle/row PE
rate as fp32r, half the SBUF/DVE traffic); projections and the output
projection stay tf32 (float32r); PSUM accumulation is fp32.

Emission is q-chunk-major ((h, qc) units ordered by qc) so the output
projection for q-chunk qc rides inside qc+1's attention stream, and the
remaining projection chunks fill early PE gaps while ACT grinds exps.
"""

import numpy as np
from collections import deque
from contextlib import ExitStack

import concourse.bass as bass
import concourse.tile as tile
from concourse import bacc, mybir
from concourse.bass_utils import run_bass_kernel_spmd
from concourse.masks import make_identity

F32 = mybir.dt.float32
F32R = mybir.dt.float32r
BF16 = mybir.dt.bfloat16
AF = mybir.ActivationFunctionType

N_CORES = 8
D_MODEL = 1024
NUM_HEADS = 16
DEPTH = 64
B_FULL = 2
S_FULL = 2048
CORES_PER_BATCH = N_CORES // B_FULL          # 4
HEADS_PER_CORE = NUM_HEADS // CORES_PER_BATCH  # 4


def build_program(T=2048, D=1024, dh=64, hc=4, with_qkv_bias=False,
                  with_o_bias=False, debug_taps=False):
    """SPMD Bass program for one core: hc heads over T tokens (one batch).

    Requires hc == 4, dh == 64, D % 128 == 0, T % 512 == 0.
    """
    d2 = hc * dh                 # 256
    npair = d2 // 128            # 2
    assert npair == 2 and dh == 64 and D % 128 == 0 and T % 512 == 0
    S = T                        # single batch per core
    ndc = D // 128               # contraction chunks
    nch = T // 512               # 512-token chunks
    KT = S // 128                # k tiles per unit
    NK2 = KT // 2                # score-pair steps per unit
    QC = S // 512                # q chunks
    NJ = min(512, D)
    npj = D // NJ                # output-proj j tiles per m tile
    scale = 1.0 / float(np.sqrt(dh))

    nc = bacc.Bacc("TRN2", target_bir_lowering=False, debug=False,
                   num_devices=N_CORES)

    xt_d = nc.dram_tensor("xt", [D, T], F32R, kind="ExternalInput").ap()
    wq_d = nc.dram_tensor("wq", [D, d2], F32R, kind="ExternalInput").ap()
    wk_d = nc.dram_tensor("wk", [D, d2], F32R, kind="ExternalInput").ap()
    wv_d = nc.dram_tensor("wv", [D, d2], F32R, kind="ExternalInput").ap()
    wo_d = nc.dram_tensor("wo", [d2, D], F32R, kind="ExternalInput").ap()
    if with_qkv_bias:
        bq_d = nc.dram_tensor("bq", [d2, 1], F32, kind="ExternalInput").ap()
        bk_d = nc.dram_tensor("bk", [d2, 1], F32, kind="ExternalInput").ap()
        bv_d = nc.dram_tensor("bv", [d2, 1], F32, kind="ExternalInput").ap()
    if with_o_bias:
        bo_d = nc.dram_tensor("bo", [1, D], F32, kind="ExternalInput").ap()
    y_d = nc.dram_tensor("y", [T, D], F32, kind="ExternalOutput").ap()

    xt_view = xt_d.rearrange("(dc p) t -> p dc t", p=128)

    with tile.TileContext(nc) as tc, ExitStack() as ctx:
        singles = ctx.enter_context(tc.tile_pool(name="singles", bufs=1))
        v2pool = ctx.enter_context(tc.tile_pool(name="v2pool", bufs=2))
        epool = ctx.enter_context(tc.tile_pool(name="epool", bufs=3))
        ysb = ctx.enter_context(tc.tile_pool(name="ysb", bufs=3))
        rcpool = ctx.enter_context(tc.tile_pool(name="rcpool", bufs=4))
        # PSUM budget (8 banks): sc 2x[128,2,512]=4, po 2x[65,512]=2,
        # ms 2x[128,512]=2
        scpool = ctx.enter_context(tc.tile_pool(name="sc", bufs=2,
                                                space="PSUM"))
        popool = ctx.enter_context(tc.tile_pool(name="po", bufs=2,
                                                space="PSUM"))
        mspool = ctx.enter_context(tc.tile_pool(name="ms", bufs=2,
                                                space="PSUM"))

        identf = singles.tile([128, 128], F32)
        make_identity(nc, identf[:])
        identb = singles.tile([128, 128], F32R)
        nc.vector.tensor_copy(identb[:], identf[:])
        ones1f = singles.tile([1, dh], F32)
        nc.vector.memset(ones1f[:], 1.0)
        ones1 = singles.tile([1, dh], F32R)
        nc.vector.tensor_copy(ones1[:], ones1f[:])

        # weights: host pre-rounded tf32 bits -> DMA straight into f32r
        # Weight DMAs are split per-dc and interleaved with the x loads in
        # need-order (emitted below) so the first projection starts in ~2us.
        w_sb = [singles.tile([128, ndc, d2], F32R, tag=n, name=n)
                for n in ("wqs", "wks", "wvs")]
        w_views = [wd.rearrange("(dc p) m -> p dc m", p=128)
                   for wd in (wq_d, wk_d, wv_d)]
        wo_sb = singles.tile([128, npair, D], F32R)

        def w_dma(kind):
            for dc in range(ndc):
                nc.sync.dma_start(out=w_sb[kind][:, dc, :],
                                  in_=w_views[kind][:, dc, :])

        # resident x; odd dc slices ride the scalar (Activation) hwdge ring,
        # even slices the sync ring, so the two rings split the 8.4MB and
        # the weights don't serialize behind x
        x_sb = singles.tile([128, ndc, T], F32R, tag="x_sb")

        b_sb = [None, None, None]
        if with_qkv_bias:
            for i, bd in enumerate((bq_d, bk_d, bv_d)):
                t = singles.tile([128, npair], F32, tag=f"b{i}", name=f"b{i}")
                nc.sync.dma_start(
                    out=t[:], in_=bd.rearrange("(pr p) one -> p (pr one)",
                                               p=128))
                b_sb[i] = t
        bo_sb = None
        if with_o_bias:
            bo_sb = singles.tile([128, D], F32)
            nc.gpsimd.dma_start(out=bo_sb[:], in_=bo_d.partition_broadcast(128))

        # attention operand tiles (bf16)
        # q2tz[h]: head h's Q^T on its pair-local 64 partitions, zeros on the
        # other 64, so the [128,128] two-head K tile contracts at K=128 with
        # the other head's contribution cancelled.
        q2tz = [singles.tile([128, T], BF16, tag=f"q2tz{h}", name=f"q2tz{h}")
                for h in range(hc)]
        for h in range(hc):
            hh = h % 2
            zrows = slice(dh, 128) if hh == 0 else slice(0, dh)
            nc.vector.memset(q2tz[h][zrows, :].bitcast(F32), 0.0)
        k2t = singles.tile([128, npair, T], BF16, tag="k2t")
        out2t = singles.tile([128, npair, T], F32R, tag="out2t")
        # vnat[:, h, kt, 0:64] = V rows (k on partitions); col 64 = ones
        vnat = singles.tile([128, hc, KT, dh + 1], BF16, tag="vnat")
        onesc = singles.tile([128, hc, KT, 1], F32)
        nc.vector.memset(onesc[:], 1.0)
        nc.vector.tensor_copy(vnat[:, :, :, dh:dh + 1], onesc[:])

        def x_dma(n, parity):
            for dc in range(parity, ndc, 2):
                eng = nc.scalar if parity else nc.sync
                eng.dma_start(
                    out=x_sb[:, dc, n * 512:(n + 1) * 512],
                    in_=xt_view[:, dc, n * 512:(n + 1) * 512])

        # ---------- emission helpers ----------
        def proj(kind, pr, n):
            """Projection of chunk n (512 tokens) for pair pr.
            kind: 0=q, 1=k, 2=v."""
            ncol = slice(n * 512, (n + 1) * 512)
            ps = mspool.tile([128, 512], F32, tag="ms", name=f"pj{kind}{pr}{n}")
            for dc in range(ndc):
                nc.tensor.matmul(ps[:], w_sb[kind][:, dc,
                                                   pr * 128:(pr + 1) * 128],
                                 x_sb[:, dc, ncol],
                                 start=(dc == 0), stop=(dc == ndc - 1))
            if kind == 0:
                for hh in range(2):
                    h = 2 * pr + hh
                    hp = slice(hh * dh, (hh + 1) * dh)
                    if with_qkv_bias:
                        nc.vector.tensor_scalar_add(
                            q2tz[h][hp, ncol], ps[hp, :],
                            b_sb[0][hp, pr:pr + 1])
                    else:
                        nc.vector.tensor_copy(q2tz[h][hp, ncol], ps[hp, :])
            elif kind == 1:
                if with_qkv_bias:
                    nc.vector.tensor_scalar_add(k2t[:, pr, ncol], ps[:],
                                                b_sb[1][:, pr:pr + 1])
                else:
                    nc.vector.tensor_copy(k2t[:, pr, ncol], ps[:])
            else:
                v2 = v2pool.tile([128, 512], F32R, tag="v2")
                if with_qkv_bias:
                    nc.vector.tensor_scalar_add(v2[:], ps[:],
                                                b_sb[2][:, pr:pr + 1])
                else:
                    nc.vector.tensor_copy(v2[:], ps[:])
                kt0 = n * 4
                pv = mspool.tile([128, 4, 128], F32R, tag="ms",
                                 name=f"pv{pr}{n}")
                for sub in range(4):
                    nc.tensor.transpose(pv[:, sub, :],
                                        v2[:, sub * 128:(sub + 1) * 128],
                                        identb[:])
                for hh in range(2):
                    h = 2 * pr + hh
                    nc.vector.tensor_copy(
                        vnat[:, h, kt0:kt0 + 4, 0:dh],
                        pv[:, :, hh * dh:(hh + 1) * dh])

        def fin2(h, qc, po, dr):
            """Broadcast the raw denominator across dh partitions with a K=1
            matmul, fast-reciprocal it, scale the numerator into out2t."""
            pr, hh = divmod(h, 2)
            hp = slice(hh * dh, (hh + 1) * dh)
            qcol = slice(qc * 512, (qc + 1) * 512)
            den = mspool.tile([dh, 512], F32, tag="ms", name=f"den{h}_{qc}")
            nc.tensor.matmul(den[:], ones1[:], dr[:], start=True, stop=True)
            rci = rcpool.tile([dh, 512], F32, tag="rci", name=f"rci{h}_{qc}")
            nc.vector.reciprocal_approx_fast(rci[:], den[:])
            nc.vector.tensor_mul(out2t[hp, pr, qcol], po[0:dh, :], rci[:])

        def p4_tile(m, j):
            """Output-projection tile: y[m*128:+128, j*NJ:+NJ]."""
            py = mspool.tile([128, NJ], F32, tag="ms", name=f"py{m}_{j}")
            for pr in range(npair):
                nc.tensor.matmul(py[:], out2t[:, pr, m * 128:(m + 1) * 128],
                                 wo_sb[:, pr, j * NJ:(j + 1) * NJ],
                                 start=(pr == 0), stop=(pr == npair - 1))
            yt = ysb.tile([128, NJ], F32, tag="yt")
            if with_o_bias:
                nc.vector.tensor_add(yt[:], py[:], bo_sb[:, j * NJ:(j + 1) * NJ])
            else:
                nc.vector.tensor_copy(yt[:], py[:])
            nc.sync.dma_start(out=y_d[m * 128:(m + 1) * 128,
                                      j * NJ:(j + 1) * NJ], in_=yt[:])

        # ---------- interleaved emission ----------
        work = deque()

        def run_item(item):
            if item[0] == "proj":
                proj(item[1], item[2], item[3])
            elif item[0] == "fin2":
                fin2(item[1], item[2], item[3], item[4])
            else:
                p4_tile(item[1], item[2])

        # prologue DMAs: odd x slices (all chunks) go to the scalar ring up
        # front; the sync ring carries weights and even x slices interleaved
        # in the order the projections need them. y DMAs reuse the sync
        # ring later (it is drained by ~30us).
        for n in range(nch):
            x_dma(n, 1)
        w_dma(1)                   # K weights first
        x_dma(0, 0)
        w_dma(0)                   # Q weights
        w_dma(2)                   # V weights
        for n in range(1, nch):
            x_dma(n, 0)
        nc.sync.dma_start(out=wo_sb[:],
                          in_=wo_d.rearrange("(pr p) m -> p pr m", p=128))

        # prologue projections: just enough to start (h0, qc0); the rest of
        # pair 0's K/V chunks are forced into unit 0's early steps to match
        # x chunk DMA arrival
        proj(1, 0, 0)              # K pair 0, chunk 0
        proj(0, 0, 0)              # Q pair 0, chunk 0
        proj(2, 0, 0)              # V pair 0, chunk 0
        forced0 = {}
        for n in range(1, nch):
            forced0[2 * (n - 1)] = [("proj", 1, 0, n), ("proj", 2, 0, n)]

        # filler queue: remaining projections in need-order, then (appended
        # as they unlock) output-projection tiles
        for n in range(nch):
            work.append(("proj", 1, 1, n))      # K p1
        work.append(("proj", 0, 1, 0))          # Q p1 chunk 0
        for n in range(nch):
            work.append(("proj", 2, 1, n))      # V p1
        for n in range(1, nch):                 # Q chunk n feeds qc == n
            work.append(("proj", 0, 0, n))
            work.append(("proj", 0, 1, n))

        units = [(h, qc) for qc in range(QC) for h in range(hc)]
        total_steps = len(units) * NK2
        # pace the queue to drain before the last unit; a small reserve of
        # qc-(QC-2) output-projection tiles fills the last unit instead
        pace_steps = max(1, total_steps - NK2)
        reserve = []
        step = 0
        for ui, (h, qc) in enumerate(units):
            last_unit = ui == len(units) - 1
            pr = h // 2
            qcol = slice(qc * 512, (qc + 1) * 512)
            po = popool.tile([dh + 1, 512], F32, tag="po", name=f"po{h}_{qc}")
            prev_ex = None
            for kt2 in range(NK2):
                sc = scpool.tile([128, 2, 512], F32, tag="sc",
                                 name=f"sc{h}_{qc}_{kt2}")
                for i in range(2):
                    kt = 2 * kt2 + i
                    kcol = slice(kt * 128, (kt + 1) * 128)
                    nc.tensor.matmul(sc[:, i, :], k2t[:, pr, kcol],
                                     q2tz[h][:, qcol], start=True, stop=True)
                ex = epool.tile([128, 2, 512], BF16, tag="ex",
                                name=f"ex{h}_{qc}_{kt2}")
                nc.scalar.activation(ex[:], sc[:], AF.Exp, scale=scale)
                if kt2 > 0:
                    for i in range(2):
                        kt = 2 * (kt2 - 1) + i
                        nc.tensor.matmul(po[:], vnat[:, h, kt, :],
                                         prev_ex[:, i, :],
                                         start=(kt == 0), stop=False)
                prev_ex = ex
                # paced filler drain (capped to keep independent PE work
                # between any queued dependency chain and its consumer)
                if ui == 0 and kt2 in forced0:
                    for item in forced0[kt2]:
                        run_item(item)
                if last_unit and reserve:
                    run_item(reserve.pop(0))
                steps_left = max(1, pace_steps - step)
                npop = min(2, (len(work) + steps_left - 1) // steps_left,
                           len(work))
                for _ in range(npop):
                    run_item(work.popleft())
                step += 1
            for i in range(2):
                kt = 2 * (NK2 - 1) + i
                nc.tensor.matmul(po[:], vnat[:, h, kt, :], prev_ex[:, i, :],
                                 start=(kt == 0), stop=(kt == KT - 1))
            # raw denominator to SBUF now (DVE, f32r-rounded for the matmul);
            # the broadcast+reciprocal+scale is queued so its latency hides
            # behind the next unit's first score matmuls
            dr = rcpool.tile([1, 512], F32R, tag="dr", name=f"dr{h}_{qc}")
            nc.vector.tensor_copy(dr[:], po[dh:dh + 1, :])
            work.appendleft(("fin2", h, qc, po, dr))
            if h == hc - 1:
                # all heads of qc done (once queued fin2s run): queue the
                # output projection for these tokens; hold back part of the
                # next-to-last q-chunk's tiles to fill the last unit
                tiles = [("p4", m, j)
                         for m in range(qc * 512 // 128, (qc + 1) * 512 // 128)
                         for j in range(npj)]
                if QC > 1 and qc == QC - 2:
                    keep = min(len(tiles), NK2 + 2)
                    reserve = tiles[-keep:]
                    tiles = tiles[:-keep]
                work.extend(tiles)
        # tail drain: the last fin2 sits at the queue front; keep reserved
        # tiles (independent PE work) between it and the final q-chunk's
        # output projection so the normalize chain latency stays hidden
        if work and work[0][0] == "fin2":
            run_item(work.popleft())
        for item in reserve:
            run_item(item)
        while work:
            run_item(work.popleft())

        if debug_taps:
            dq = nc.dram_tensor("dbg_q", [128, hc, T], F32,
                                kind="ExternalOutput").ap()
            dk = nc.dram_tensor("dbg_k", [128, npair, T], F32,
                                kind="ExternalOutput").ap()
            dv = nc.dram_tensor("dbg_v", [128, hc, KT, dh + 1], F32,
                                kind="ExternalOutput").ap()
            do = nc.dram_tensor("dbg_o", [128, npair, T], F32,
                                kind="ExternalOutput").ap()
            with tc.tile_pool(name="dbg", bufs=1) as dbg:
                tq = dbg.tile([128, hc, T], F32)
                for h in range(hc):
                    nc.vector.tensor_copy(tq[:, h, :], q2tz[h][:])
                nc.sync.dma_start(out=dq, in_=tq[:])
                tk = dbg.tile([128, npair, T], F32)
                nc.vector.tensor_copy(tk[:], k2t[:])
                nc.sync.dma_start(out=dk, in_=tk[:])
                tv = dbg.tile([128, hc, KT, dh + 1], F32)
                nc.vector.tensor_copy(tv[:], vnat[:])
                nc.sync.dma_start(out=dv, in_=tv[:])
                to = dbg.tile([128, npair, T], F32)
                nc.vector.tensor_copy(to[:], out2t[:])
                nc.sync.dma_start(out=do, in_=to[:])

    nc.compile()
    return nc


_PROGRAM_CACHE = {}


def _get_program(key):
    if key not in _PROGRAM_CACHE:
        with_qkv_bias, with_o_bias = key
        _PROGRAM_CACHE[key] = build_program(
            with_qkv_bias=with_qkv_bias, with_o_bias=with_o_bias)
    return _PROGRAM_CACHE[key]


def _round_tf32(a):
    """Round fp32 to tf32 (10-bit mantissa), round-to-nearest-even."""
    u = np.ascontiguousarray(a).view(np.uint32)
    r = (u + 0xFFF + ((u >> 13) & 1)) & np.uint32(0xFFFFE000)
    return r.view(np.float32)


def make_in_maps(x, wq, bq, wk, bk, wv, bv, wo, bo, with_qkv_bias, with_o_bias,
                 S=S_FULL, hc=HEADS_PER_CORE, dh=DEPTH):
    """x: [B*S, D] fp32. Core c gets batch c // CORES_PER_BATCH and head
    columns d2*(c % CORES_PER_BATCH) .. +d2."""
    d2 = hc * dh
    nb = x.shape[0] // S
    xts = [_round_tf32(np.ascontiguousarray(x[b * S:(b + 1) * S].T))
           for b in range(nb)]
    wqr, wkr, wvr, wor = (_round_tf32(w) for w in (wq, wk, wv, wo))
    in_maps = []
    for c in range(N_CORES):
        g = c % CORES_PER_BATCH
        cs = slice(g * d2, (g + 1) * d2)
        m = {"xt": xts[c // CORES_PER_BATCH],
             "wq": np.ascontiguousarray(wqr[:, cs]),
             "wk": np.ascontiguousarray(wkr[:, cs]),
             "wv": np.ascontiguousarray(wvr[:, cs]),
             "wo": np.ascontiguousarray(wor[cs, :])}
        if with_qkv_bias:
            m["bq"] = np.ascontiguousarray(bq[cs].reshape(d2, 1))
            m["bk"] = np.ascontiguousarray(bk[cs].reshape(d2, 1))
            m["bv"] = np.ascontiguousarray(bv[cs].reshape(d2, 1))
        if with_o_bias:
            m["bo"] = (bo.reshape(1, -1).astype(np.float32) if g == 0
                       else np.zeros((1, bo.shape[-1]), np.float32))
        in_maps.append(m)
    return in_maps


def kernel(inputs, wq, bq, wk, bk, wv, bv, wo, bo):
    x = np.ascontiguousarray(np.asarray(inputs, np.float32)
                             .reshape(B_FULL * S_FULL, D_MODEL))
    wq, wk, wv, wo = (np.asarray(a, np.float32) for a in (wq, wk, wv, wo))
    bq, bk, bv, bo = (np.asarray(a, np.float32) for a in (bq, bk, bv, bo))

    with_qkv_bias = bool(np.any(bq) or np.any(bk) or np.any(bv))
    with_o_bias = bool(np.any(bo))
    nc = _get_program((with_qkv_bias, with_o_bias))

    in_maps = make_in_maps(x, wq, bq, wk, bk, wv, bv, wo, bo,
                           with_qkv_bias, with_o_bias)
    res = run_bass_kernel_spmd(nc, in_maps, list(range(N_CORES))).results
    y = np.zeros((B_FULL, S_FULL, D_MODEL), np.float64)
    for c in range(N_CORES):
        y[c // CORES_PER_BATCH] += res[c]["y"]
    return y.astype(np.float32)


# revision 23
# speedup vs baseline: 1.0680x; 1.0680x over previous
"""Multi-head self-attention (B=2, S=2048, D=1024, H=16) on 8 TRN2 NeuronCores.

Sharding: batch x head-group parallel. Core c handles batch c//4 and the
4 heads 4*(c%4)..4*(c%4)+3. Each core reads only its batch's tokens,
computes Q/K/V for its 4 heads, full non-causal softmax attention for its
4 (head) units, and the partial output projection y_c = sum_h out_h@wo[h].
Host sums the 4 partials per batch. Host pre-transposes x to xT and
pre-rounds x and all weights to bf16 so the head of the kernel (which is
HBM-bound on the x + weight loads) moves half the bytes.

Device dataflow per core (heads grouped in 2 pairs of 2; head-dim on
partitions; pair p owns partitions [0:64] for head 2p, [64:128] for 2p+1):
  k2t[p] / q2tz[h] / out2t[p] [128, T]; x resident in SBUF [128, ndc, T]
  proj: psum = sum_dc w[:,dc,p*128:+128]^T @ x[:,dc,chunk]   (PSUM accum)
  v --PE transpose--> vnat[h] [k, 64] bf16 (+ ones column -> denominator)
  scoresT[k, q] = K Q^T with per-head zero-padded Q (K=128 contraction);
    score matmuls write PAIRS of PSUM banks, one ACT exp instruction
    covers both ([128, 1024]) to amortize ACT fixed overheads
  poT[65, q] += vnat[h,kt]^T @ exp  accumulated over kt (PSUM)
  normalize: dr=[1,512] raw denominator (f32r) -> K=1 matmul broadcast ->
    reciprocal_approx_fast -> out2t[hp, q] = po[0:64] * rcp (DVE)
  y[s, :] = sum_p out2t[p][:, s-tile]^T @ wo[p]  -> SBUF -> DMA

All matmul operands are bf16 except the f32r denominator broadcast and
the f32r V-transpose path; PSUM accumulation is fp32. bf16 and f32r both
stream 1 PE row/cycle, so this costs no PE time, only accuracy that stays
well inside the 2e-2 budget.

Scheduling: q-chunk-major (h, qc) units; exp runs on pairs of k-tiles.
During attention the ACT engine is the local pacer, so all filler work
(remaining projections, output-projection tiles, normalizations) is
emitted through generators that yield every ~2 matmuls, weaving at most a
few hundred ns of PE work between consecutive score pairs — a coarse
filler burst would starve ACT. x DMAs split across the sync and scalar
hwdge rings (the two rings share HBM but halve serialization); y writes
alternate sync/gpsimd rings so the tail is not bound by one ring.
"""

import numpy as np
from collections import deque
from contextlib import ExitStack

import ml_dtypes
import concourse.bass as bass
import concourse.tile as tile
from concourse import bacc, mybir
from concourse.bass_utils import run_bass_kernel_spmd
from concourse.masks import make_identity

F32 = mybir.dt.float32
F32R = mybir.dt.float32r
BF16 = mybir.dt.bfloat16
AF = mybir.ActivationFunctionType

N_CORES = 8
D_MODEL = 1024
NUM_HEADS = 16
DEPTH = 64
B_FULL = 2
S_FULL = 2048
CORES_PER_BATCH = N_CORES // B_FULL          # 4
HEADS_PER_CORE = NUM_HEADS // CORES_PER_BATCH  # 4


def build_program(T=2048, D=1024, dh=64, hc=4, with_qkv_bias=False,
                  with_o_bias=False, debug_taps=False, bf16_io=True):
    """SPMD Bass program for one core: hc heads over T tokens (one batch).

    Requires hc == 4, dh == 64, D % 128 == 0, T % 512 == 0.
    """
    d2 = hc * dh                 # 256
    npair = d2 // 128            # 2
    assert npair == 2 and dh == 64 and D % 128 == 0 and T % 512 == 0
    S = T                        # single batch per core
    ndc = D // 128               # contraction chunks
    nch = T // 512               # 512-token chunks
    KT = S // 128                # k tiles per unit
    NK2 = KT // 2                # score-pair steps per unit
    QC = S // 512                # q chunks
    NJ = min(512, D)
    npj = D // NJ                # output-proj j tiles per m tile
    scale = 1.0 / float(np.sqrt(dh))

    nc = bacc.Bacc("TRN2", target_bir_lowering=False, debug=False,
                   num_devices=N_CORES)
    IODT = BF16 if bf16_io else F32R

    xt_d = nc.dram_tensor("xt", [D, T], IODT, kind="ExternalInput").ap()
    wq_d = nc.dram_tensor("wq", [D, d2], IODT, kind="ExternalInput").ap()
    wk_d = nc.dram_tensor("wk", [D, d2], IODT, kind="ExternalInput").ap()
    wv_d = nc.dram_tensor("wv", [D, d2], IODT, kind="ExternalInput").ap()
    wo_d = nc.dram_tensor("wo", [d2, D], IODT, kind="ExternalInput").ap()
    if with_qkv_bias:
        bq_d = nc.dram_tensor("bq", [d2, 1], F32, kind="ExternalInput").ap()
        bk_d = nc.dram_tensor("bk", [d2, 1], F32, kind="ExternalInput").ap()
        bv_d = nc.dram_tensor("bv", [d2, 1], F32, kind="ExternalInput").ap()
    if with_o_bias:
        bo_d = nc.dram_tensor("bo", [1, D], F32, kind="ExternalInput").ap()
    y_d = nc.dram_tensor("y", [T, D], F32, kind="ExternalOutput").ap()

    xt_view = xt_d.rearrange("(dc p) t -> p dc t", p=128)

    with tile.TileContext(nc) as tc, ExitStack() as ctx:
        singles = ctx.enter_context(tc.tile_pool(name="singles", bufs=1))
        v2pool = ctx.enter_context(tc.tile_pool(name="v2pool", bufs=2))
        epool = ctx.enter_context(tc.tile_pool(name="epool", bufs=3))
        ysb = ctx.enter_context(tc.tile_pool(name="ysb", bufs=4))
        rcpool = ctx.enter_context(tc.tile_pool(name="rcpool", bufs=4))
        # PSUM budget (8 banks): sc 2x[128,2,512]=4, po 2x[65,512]=2,
        # ms 2x[128,512]=2
        scpool = ctx.enter_context(tc.tile_pool(name="sc", bufs=2,
                                                space="PSUM"))
        popool = ctx.enter_context(tc.tile_pool(name="po", bufs=2,
                                                space="PSUM"))
        mspool = ctx.enter_context(tc.tile_pool(name="ms", bufs=2,
                                                space="PSUM"))

        identf = singles.tile([128, 128], F32)
        make_identity(nc, identf[:])
        identb = singles.tile([128, 128], F32R)
        nc.vector.tensor_copy(identb[:], identf[:])
        ones1f = singles.tile([1, dh], F32)
        nc.vector.memset(ones1f[:], 1.0)
        ones1 = singles.tile([1, dh], F32R)
        nc.vector.tensor_copy(ones1[:], ones1f[:])

        # Weight DMAs split per-dc, interleaved with x loads in need-order
        w_sb = [singles.tile([128, ndc, d2], IODT, tag=n, name=n)
                for n in ("wqs", "wks", "wvs")]
        w_views = [wd.rearrange("(dc p) m -> p dc m", p=128)
                   for wd in (wq_d, wk_d, wv_d)]
        wo_sb = singles.tile([128, npair, D], IODT)

        def w_dma(kind):
            for dc in range(ndc):
                nc.sync.dma_start(out=w_sb[kind][:, dc, :],
                                  in_=w_views[kind][:, dc, :])

        # resident x; odd dc slices ride the scalar (Activation) hwdge ring,
        # even slices the sync ring
        x_sb = singles.tile([128, ndc, T], IODT, tag="x_sb")

        def x_dma(n, parity):
            for dc in range(parity, ndc, 2):
                eng = nc.scalar if parity else nc.sync
                eng.dma_start(
                    out=x_sb[:, dc, n * 512:(n + 1) * 512],
                    in_=xt_view[:, dc, n * 512:(n + 1) * 512])

        b_sb = [None, None, None]
        if with_qkv_bias:
            for i, bd in enumerate((bq_d, bk_d, bv_d)):
                t = singles.tile([128, npair], F32, tag=f"b{i}", name=f"b{i}")
                nc.sync.dma_start(
                    out=t[:], in_=bd.rearrange("(pr p) one -> p (pr one)",
                                               p=128))
                b_sb[i] = t
        bo_sb = None
        if with_o_bias:
            bo_sb = singles.tile([128, D], F32)
            nc.gpsimd.dma_start(out=bo_sb[:], in_=bo_d.partition_broadcast(128))

        # attention operand tiles (bf16)
        # q2tz[h]: head h's Q^T on its pair-local 64 partitions, zeros on the
        # other 64, so the [128,128] two-head K tile contracts at K=128 with
        # the other head's contribution cancelled.
        q2tz = [singles.tile([128, T], BF16, tag=f"q2tz{h}", name=f"q2tz{h}")
                for h in range(hc)]
        for h in range(hc):
            hh = h % 2
            zrows = slice(dh, 128) if hh == 0 else slice(0, dh)
            nc.vector.memset(q2tz[h][zrows, :].bitcast(F32), 0.0)
        k2t = singles.tile([128, npair, T], BF16, tag="k2t")
        out2t = singles.tile([128, npair, T], IODT, tag="out2t")
        # vnat[:, h, kt, 0:64] = V rows (k on partitions); col 64 = ones
        vnat = singles.tile([128, hc, KT, dh + 1], BF16, tag="vnat")
        onesc = singles.tile([128, hc, KT, 1], F32)
        nc.vector.memset(onesc[:], 1.0)
        nc.vector.tensor_copy(vnat[:, :, :, dh:dh + 1], onesc[:])

        # ---------- filler generators ----------
        # Each yields between ~2-matmul chunks of PE work so the drain loop
        # can weave them between score pairs without starving ACT. Only one
        # generator is open at a time (they share the 2-buf ms PSUM pool).
        def proj_gen(kind, pr, n):
            """Projection of chunk n (512 tokens) for pair pr.
            kind: 0=q, 1=k, 2=v."""
            ncol = slice(n * 512, (n + 1) * 512)
            ps = mspool.tile([128, 512], F32, tag="ms", name=f"pj{kind}{pr}{n}")
            for dc in range(ndc):
                nc.tensor.matmul(ps[:], w_sb[kind][:, dc,
                                                   pr * 128:(pr + 1) * 128],
                                 x_sb[:, dc, ncol],
                                 start=(dc == 0), stop=(dc == ndc - 1))
                if dc % 2 == 1 and dc < ndc - 1:
                    yield
            if kind == 0:
                for hh in range(2):
                    h = 2 * pr + hh
                    hp = slice(hh * dh, (hh + 1) * dh)
                    if with_qkv_bias:
                        nc.vector.tensor_scalar_add(
                            q2tz[h][hp, ncol], ps[hp, :],
                            b_sb[0][hp, pr:pr + 1])
                    else:
                        nc.vector.tensor_copy(q2tz[h][hp, ncol], ps[hp, :])
            elif kind == 1:
                if with_qkv_bias:
                    nc.vector.tensor_scalar_add(k2t[:, pr, ncol], ps[:],
                                                b_sb[1][:, pr:pr + 1])
                else:
                    nc.vector.tensor_copy(k2t[:, pr, ncol], ps[:])
            else:
                v2 = v2pool.tile([128, 512], F32R, tag="v2")
                if with_qkv_bias:
                    nc.vector.tensor_scalar_add(v2[:], ps[:],
                                                b_sb[2][:, pr:pr + 1])
                else:
                    nc.vector.tensor_copy(v2[:], ps[:])
                yield
                kt0 = n * 4
                pv = mspool.tile([128, 4, 128], F32R, tag="ms",
                                 name=f"pv{pr}{n}")
                for sub in range(4):
                    nc.tensor.transpose(pv[:, sub, :],
                                        v2[:, sub * 128:(sub + 1) * 128],
                                        identb[:])
                for hh in range(2):
                    h = 2 * pr + hh
                    nc.vector.tensor_copy(
                        vnat[:, h, kt0:kt0 + 4, 0:dh],
                        pv[:, :, hh * dh:(hh + 1) * dh])

        def fin2_gen(h, qc, po, dr):
            """Broadcast the raw denominator across dh partitions with a K=1
            matmul, fast-reciprocal it, scale the numerator into out2t."""
            pr, hh = divmod(h, 2)
            hp = slice(hh * dh, (hh + 1) * dh)
            qcol = slice(qc * 512, (qc + 1) * 512)
            den = mspool.tile([dh, 512], F32, tag="ms", name=f"den{h}_{qc}")
            nc.tensor.matmul(den[:], ones1[:], dr[:], start=True, stop=True)
            rci = rcpool.tile([dh, 512], F32, tag="rci", name=f"rci{h}_{qc}")
            nc.vector.reciprocal_approx_fast(rci[:], den[:])
            nc.vector.tensor_mul(out2t[hp, pr, qcol], po[0:dh, :], rci[:])
            return
            yield  # pragma: no cover (make this a generator)

        def p4_gen(m, j):
            """Output-projection tile: y[m*128:+128, j*NJ:+NJ]."""
            py = mspool.tile([128, NJ], F32, tag="ms", name=f"py{m}_{j}")
            nc.tensor.matmul(py[:], out2t[:, 0, m * 128:(m + 1) * 128],
                             wo_sb[:, 0, j * NJ:(j + 1) * NJ],
                             start=True, stop=(npair == 1))
            yield
            for pr in range(1, npair):
                nc.tensor.matmul(py[:], out2t[:, pr, m * 128:(m + 1) * 128],
                                 wo_sb[:, pr, j * NJ:(j + 1) * NJ],
                                 start=False, stop=(pr == npair - 1))
            yt = ysb.tile([128, NJ], F32, tag="yt")
            if with_o_bias:
                nc.vector.tensor_add(yt[:], py[:],
                                     bo_sb[:, j * NJ:(j + 1) * NJ])
            else:
                nc.vector.tensor_copy(yt[:], py[:])
            nc.sync.dma_start(out=y_d[m * 128:(m + 1) * 128,
                                      j * NJ:(j + 1) * NJ], in_=yt[:])

        # ---------- interleaved emission ----------
        work = deque()       # items: ("proj",k,pr,n) ("fin2",...) ("p4",m,j)
        EST = {"proj": 5, "fin2": 1, "p4": 2}
        state = {"open": None}
        done = set()         # proj items already emitted (dedup for ensure)

        def make_gen(item):
            if item[0] == "proj":
                return proj_gen(item[1], item[2], item[3])
            if item[0] == "fin2":
                return fin2_gen(item[1], item[2], item[3], item[4])
            return p4_gen(item[1], item[2])

        def pop_item():
            while work:
                item = work.popleft()
                if item[0] == "proj":
                    if item in done:
                        continue
                    done.add(item)
                return item
            return None

        def tick():
            """Advance the open generator by one segment (~2 matmuls)."""
            if state["open"] is None:
                item = pop_item()
                if item is None:
                    return False
                state["open"] = make_gen(item)
            try:
                next(state["open"])
            except StopIteration:
                state["open"] = None
            return True

        def drain_all():
            while state["open"] is not None or work:
                tick()

        def run_now(item):
            """Finish any open generator, then run item to completion."""
            while state["open"] is not None:
                tick()
            if item[0] == "proj":
                if item in done:
                    return
                done.add(item)
            state["open"] = make_gen(item)
            while state["open"] is not None:
                tick()

        def ensure_proj(kind, pr, n):
            run_now(("proj", kind, pr, n))

        # prologue DMAs: odd x slices to the scalar ring up front; sync ring
        # carries weights and even x slices in need-order, then y later
        for n in range(nch):
            x_dma(n, 1)
        w_dma(1)                   # K weights first
        x_dma(0, 0)
        w_dma(0)                   # Q weights
        w_dma(2)                   # V weights
        for n in range(1, nch):
            x_dma(n, 0)
        nc.sync.dma_start(out=wo_sb[:],
                          in_=wo_d.rearrange("(pr p) m -> p pr m", p=128))

        # prologue projections: just enough to start (h0, qc0); later
        # chunks are pulled just-in-time by ensure_proj inside the units
        for it in (("proj", 1, 0, 0), ("proj", 0, 0, 0), ("proj", 2, 0, 0)):
            run_now(it)
        for n in range(nch):
            work.append(("proj", 1, 1, n))      # K p1
        work.append(("proj", 0, 1, 0))          # Q p1 chunk 0
        for n in range(nch):
            work.append(("proj", 2, 1, n))      # V p1
        for n in range(1, nch):
            work.append(("proj", 1, 0, n))      # K p0 rest
            work.append(("proj", 2, 0, n))      # V p0 rest
        for n in range(1, nch):                 # Q chunk n feeds qc == n
            work.append(("proj", 0, 0, n))
            work.append(("proj", 0, 1, n))

        units = [(h, qc) for qc in range(QC) for h in range(hc)]
        total_steps = len(units) * NK2
        pace_steps = max(1, total_steps - NK2)
        reserve = []
        step = 0
        for ui, (h, qc) in enumerate(units):
            last_unit = ui == len(units) - 1
            pr = h // 2
            qcol = slice(qc * 512, (qc + 1) * 512)
            po = popool.tile([dh + 1, 512], F32, tag="po", name=f"po{h}_{qc}")
            ensure_proj(0, pr, qc)
            prev_ex = None
            for kt2 in range(NK2):
                # just-in-time: this step's score k-tiles and po v-tiles
                ch = (2 * kt2 + 1) // 4
                ensure_proj(1, pr, ch)
                ensure_proj(2, pr, ch)
                sc = scpool.tile([128, 2, 512], F32, tag="sc",
                                 name=f"sc{h}_{qc}_{kt2}")
                for i in range(2):
                    kt = 2 * kt2 + i
                    kcol = slice(kt * 128, (kt + 1) * 128)
                    nc.tensor.matmul(sc[:, i, :], k2t[:, pr, kcol],
                                     q2tz[h][:, qcol], start=True, stop=True)
                ex = epool.tile([128, 2, 512], BF16, tag="ex",
                                name=f"ex{h}_{qc}_{kt2}")
                nc.scalar.activation(ex[:], sc[:], AF.Exp, scale=scale)
                if kt2 > 0:
                    for i in range(2):
                        kt = 2 * (kt2 - 1) + i
                        nc.tensor.matmul(po[:], vnat[:, h, kt, :],
                                         prev_ex[:, i, :],
                                         start=(kt == 0), stop=False)
                prev_ex = ex
                # weave filler segments, paced to drain before the last unit
                if last_unit and reserve:
                    run_now(reserve.pop(0))
                est = sum(EST[it[0]] for it in work)
                steps_left = max(1, pace_steps - step)
                nticks = min(3, max(0, (est + steps_left - 1) // steps_left)
                             if (work or state["open"]) else 0)
                if state["open"] is not None and nticks == 0:
                    nticks = 1
                for _ in range(nticks):
                    tick()
                step += 1
            for i in range(2):
                kt = 2 * (NK2 - 1) + i
                nc.tensor.matmul(po[:], vnat[:, h, kt, :], prev_ex[:, i, :],
                                 start=(kt == 0), stop=(kt == KT - 1))
            # raw denominator to SBUF now (f32r for the broadcast matmul);
            # the broadcast+reciprocal+scale chain is queued so its latency
            # hides behind the next unit's first score matmuls
            dr = rcpool.tile([1, 512], F32R, tag="dr", name=f"dr{h}_{qc}")
            nc.vector.tensor_copy(dr[:], po[dh:dh + 1, :])
            work.appendleft(("fin2", h, qc, po, dr))
            if h == hc - 1:
                tiles = [("p4", m, j)
                         for m in range(qc * 512 // 128, (qc + 1) * 512 // 128)
                         for j in range(npj)]
                if QC > 1 and qc == QC - 2:
                    keep = min(len(tiles), NK2 // 2 + 2)
                    reserve = tiles[-keep:]
                    tiles = tiles[:-keep]
                work.extend(tiles)
        # tail drain: run the final fin2 first, then the reserved tiles
        # (independent PE work) to hide the normalize chain, then the rest
        if work and work[0][0] == "fin2":
            run_now(work.popleft())
        work.extendleft(reversed(reserve))
        drain_all()

        if debug_taps:
            dq = nc.dram_tensor("dbg_q", [128, hc, T], F32,
                                kind="ExternalOutput").ap()
            dk = nc.dram_tensor("dbg_k", [128, npair, T], F32,
                                kind="ExternalOutput").ap()
            dv = nc.dram_tensor("dbg_v", [128, hc, KT, dh + 1], F32,
                                kind="ExternalOutput").ap()
            do = nc.dram_tensor("dbg_o", [128, npair, T], F32,
                                kind="ExternalOutput").ap()
            with tc.tile_pool(name="dbg", bufs=1) as dbg:
                tq = dbg.tile([128, hc, T], F32)
                for h in range(hc):
                    nc.vector.tensor_copy(tq[:, h, :], q2tz[h][:])
                nc.sync.dma_start(out=dq, in_=tq[:])
                tk = dbg.tile([128, npair, T], F32)
                nc.vector.tensor_copy(tk[:], k2t[:])
                nc.sync.dma_start(out=dk, in_=tk[:])
                tv = dbg.tile([128, hc, KT, dh + 1], F32)
                nc.vector.tensor_copy(tv[:], vnat[:])
                nc.sync.dma_start(out=dv, in_=tv[:])
                to = dbg.tile([128, npair, T], F32)
                nc.vector.tensor_copy(to[:], out2t[:])
                nc.sync.dma_start(out=do, in_=to[:])

    nc.compile()
    return nc


_PROGRAM_CACHE = {}


def _get_program(key):
    if key not in _PROGRAM_CACHE:
        with_qkv_bias, with_o_bias = key
        _PROGRAM_CACHE[key] = build_program(
            with_qkv_bias=with_qkv_bias, with_o_bias=with_o_bias,
            bf16_io=BF16_IO)
    return _PROGRAM_CACHE[key]


def _round_tf32(a):
    """Round fp32 to tf32 (10-bit mantissa), round-to-nearest-even."""
    u = np.ascontiguousarray(a).view(np.uint32)
    r = (u + 0xFFF + ((u >> 13) & 1)) & np.uint32(0xFFFFE000)
    return r.view(np.float32)


BF16_IO = True


def _to_bf16(a):
    """fp32 -> bf16 (RTNE), or tf32-rounded fp32 when BF16_IO is off."""
    if BF16_IO:
        return np.ascontiguousarray(a).astype(ml_dtypes.bfloat16)
    return _round_tf32(np.ascontiguousarray(a, np.float32))


def make_in_maps(x, wq, bq, wk, bk, wv, bv, wo, bo, with_qkv_bias, with_o_bias,
                 S=S_FULL, hc=HEADS_PER_CORE, dh=DEPTH):
    """x: [B*S, D] fp32. Core c gets batch c // CORES_PER_BATCH and head
    columns d2*(c % CORES_PER_BATCH) .. +d2."""
    d2 = hc * dh
    nb = x.shape[0] // S
    xts = [_to_bf16(x[b * S:(b + 1) * S].T) for b in range(nb)]
    wqh, wkh, wvh, woh = (_to_bf16(w) for w in (wq, wk, wv, wo))
    in_maps = []
    for c in range(N_CORES):
        g = c % CORES_PER_BATCH
        cs = slice(g * d2, (g + 1) * d2)
        m = {"xt": xts[c // CORES_PER_BATCH],
             "wq": np.ascontiguousarray(wqh[:, cs]),
             "wk": np.ascontiguousarray(wkh[:, cs]),
             "wv": np.ascontiguousarray(wvh[:, cs]),
             "wo": np.ascontiguousarray(woh[cs, :])}
        if with_qkv_bias:
            m["bq"] = np.ascontiguousarray(bq[cs].reshape(d2, 1))
            m["bk"] = np.ascontiguousarray(bk[cs].reshape(d2, 1))
            m["bv"] = np.ascontiguousarray(bv[cs].reshape(d2, 1))
        if with_o_bias:
            m["bo"] = (bo.reshape(1, -1).astype(np.float32) if g == 0
                       else np.zeros((1, bo.shape[-1]), np.float32))
        in_maps.append(m)
    return in_maps


def kernel(inputs, wq, bq, wk, bk, wv, bv, wo, bo):
    x = np.ascontiguousarray(np.asarray(inputs, np.float32)
                             .reshape(B_FULL * S_FULL, D_MODEL))
    wq, wk, wv, wo = (np.asarray(a, np.float32) for a in (wq, wk, wv, wo))
    bq, bk, bv, bo = (np.asarray(a, np.float32) for a in (bq, bk, bv, bo))

    with_qkv_bias = bool(np.any(bq) or np.any(bk) or np.any(bv))
    with_o_bias = bool(np.any(bo))
    nc = _get_program((with_qkv_bias, with_o_bias))

    in_maps = make_in_maps(x, wq, bq, wk, bk, wv, bv, wo, bo,
                           with_qkv_bias, with_o_bias)
    res = run_bass_kernel_spmd(nc, in_maps, list(range(N_CORES))).results
    y = np.zeros((B_FULL, S_FULL, D_MODEL), np.float64)
    for c in range(N_CORES):
        y[c // CORES_PER_BATCH] += res[c]["y"]
    return y.astype(np.float32)


# revision 24
# speedup vs baseline: 1.0738x; 1.0055x over previous
"""Multi-head self-attention (B=2, S=2048, D=1024, H=16) on 8 TRN2 NeuronCores.

Sharding: batch x head-group parallel. Core c handles batch c//4 and the
4 heads 4*(c%4)..4*(c%4)+3. Each core reads only its batch's tokens,
computes Q/K/V for its 4 heads, full non-causal softmax attention for its
4 (head) units, and the partial output projection y_c = sum_h out_h@wo[h].
Host sums the 4 partials per batch. Host pre-transposes x to xT and
pre-rounds x and all weights to bf16 so the head of the kernel (which is
HBM-bound on the x + weight loads) moves half the bytes.

Device dataflow per core (heads grouped in 2 pairs of 2; head-dim on
partitions; pair p owns partitions [0:64] for head 2p, [64:128] for 2p+1):
  k2t[p] / q2tz[h] / out2t[p] [128, T]; x resident in SBUF [128, ndc, T]
  proj: psum = sum_dc w[:,dc,p*128:+128]^T @ x[:,dc,chunk]   (PSUM accum)
  v --PE transpose--> vnat[h] [k, 64] bf16 (+ ones column -> denominator)
  scoresT[k, q] = K Q^T with per-head zero-padded Q (K=128 contraction);
    score matmuls write PAIRS of PSUM banks, one ACT exp instruction
    covers both ([128, 1024]) to amortize ACT fixed overheads
  poT[65, q] += vnat[h,kt]^T @ exp  accumulated over kt (PSUM)
  normalize: dr=[1,512] raw denominator (f32r) -> K=1 matmul broadcast ->
    reciprocal_approx_fast -> out2t[hp, q] = po[0:64] * rcp (DVE)
  y[s, :] = sum_p out2t[p][:, s-tile]^T @ wo[p]  -> SBUF -> DMA

All matmul operands are bf16 except the f32r denominator broadcast and
the f32r V-transpose path; PSUM accumulation is fp32. bf16 and f32r both
stream 1 PE row/cycle, so this costs no PE time, only accuracy that stays
well inside the 2e-2 budget.

Scheduling: q-chunk-major (h, qc) units; exp runs on pairs of k-tiles.
During attention the ACT engine is the local pacer, so all filler work
(remaining projections, output-projection tiles, normalizations) is
emitted through generators that yield every ~2 matmuls, weaving at most a
few hundred ns of PE work between consecutive score pairs — a coarse
filler burst would starve ACT. x DMAs split across the sync and scalar
hwdge rings (the two rings share HBM but halve serialization); y writes
alternate sync/gpsimd rings so the tail is not bound by one ring.
"""

import numpy as np
from collections import deque
from contextlib import ExitStack

import ml_dtypes
import concourse.bass as bass
import concourse.tile as tile
from concourse import bacc, mybir
from concourse.bass_utils import run_bass_kernel_spmd
from concourse.masks import make_identity

F32 = mybir.dt.float32
F32R = mybir.dt.float32r
BF16 = mybir.dt.bfloat16
AF = mybir.ActivationFunctionType

N_CORES = 8
D_MODEL = 1024
NUM_HEADS = 16
DEPTH = 64
B_FULL = 2
S_FULL = 2048
CORES_PER_BATCH = N_CORES // B_FULL          # 4
HEADS_PER_CORE = NUM_HEADS // CORES_PER_BATCH  # 4


def build_program(T=2048, D=1024, dh=64, hc=4, with_qkv_bias=False,
                  with_o_bias=False, debug_taps=False, bf16_io=True):
    """SPMD Bass program for one core: hc heads over T tokens (one batch).

    Requires hc == 4, dh == 64, D % 128 == 0, T % 512 == 0.
    """
    d2 = hc * dh                 # 256
    npair = d2 // 128            # 2
    assert npair == 2 and dh == 64 and D % 128 == 0 and T % 512 == 0
    S = T                        # single batch per core
    ndc = D // 128               # contraction chunks
    nch = T // 512               # 512-token chunks
    KT = S // 128                # k tiles per unit
    NK2 = KT // 2                # score-pair steps per unit
    QC = S // 512                # q chunks
    NJ = min(512, D)
    npj = D // NJ                # output-proj j tiles per m tile
    scale = 1.0 / float(np.sqrt(dh))

    nc = bacc.Bacc("TRN2", target_bir_lowering=False, debug=False,
                   num_devices=N_CORES)
    IODT = BF16 if bf16_io else F32R

    xt_d = nc.dram_tensor("xt", [D, T], IODT, kind="ExternalInput").ap()
    wq_d = nc.dram_tensor("wq", [D, d2], IODT, kind="ExternalInput").ap()
    wk_d = nc.dram_tensor("wk", [D, d2], IODT, kind="ExternalInput").ap()
    wv_d = nc.dram_tensor("wv", [D, d2], IODT, kind="ExternalInput").ap()
    wo_d = nc.dram_tensor("wo", [d2, D], IODT, kind="ExternalInput").ap()
    if with_qkv_bias:
        bq_d = nc.dram_tensor("bq", [d2, 1], F32, kind="ExternalInput").ap()
        bk_d = nc.dram_tensor("bk", [d2, 1], F32, kind="ExternalInput").ap()
        bv_d = nc.dram_tensor("bv", [d2, 1], F32, kind="ExternalInput").ap()
    if with_o_bias:
        bo_d = nc.dram_tensor("bo", [1, D], F32, kind="ExternalInput").ap()
    y_d = nc.dram_tensor("y", [T, D], BF16, kind="ExternalOutput").ap()

    xt_view = xt_d.rearrange("(dc p) t -> p dc t", p=128)

    with tile.TileContext(nc) as tc, ExitStack() as ctx:
        singles = ctx.enter_context(tc.tile_pool(name="singles", bufs=1))
        v2pool = ctx.enter_context(tc.tile_pool(name="v2pool", bufs=2))
        epool = ctx.enter_context(tc.tile_pool(name="epool", bufs=3))
        ysb = ctx.enter_context(tc.tile_pool(name="ysb", bufs=4))
        rcpool = ctx.enter_context(tc.tile_pool(name="rcpool", bufs=4))
        # PSUM budget (8 banks): sc 2x[128,2,512]=4, po 2x[65,512]=2,
        # ms 2x[128,512]=2
        scpool = ctx.enter_context(tc.tile_pool(name="sc", bufs=2,
                                                space="PSUM"))
        popool = ctx.enter_context(tc.tile_pool(name="po", bufs=2,
                                                space="PSUM"))
        mspool = ctx.enter_context(tc.tile_pool(name="ms", bufs=2,
                                                space="PSUM"))

        identf = singles.tile([128, 128], F32)
        make_identity(nc, identf[:])
        identb = singles.tile([128, 128], F32R)
        nc.vector.tensor_copy(identb[:], identf[:])
        ones1f = singles.tile([1, dh], F32)
        nc.vector.memset(ones1f[:], 1.0)
        ones1 = singles.tile([1, dh], F32R)
        nc.vector.tensor_copy(ones1[:], ones1f[:])

        # Weight DMAs split per-dc, interleaved with x loads in need-order
        w_sb = [singles.tile([128, ndc, d2], IODT, tag=n, name=n)
                for n in ("wqs", "wks", "wvs")]
        w_views = [wd.rearrange("(dc p) m -> p dc m", p=128)
                   for wd in (wq_d, wk_d, wv_d)]
        wo_sb = singles.tile([128, npair, D], IODT)

        def w_dma(kind):
            for dc in range(ndc):
                nc.sync.dma_start(out=w_sb[kind][:, dc, :],
                                  in_=w_views[kind][:, dc, :])

        # resident x; odd dc slices ride the scalar (Activation) hwdge ring,
        # even slices the sync ring
        x_sb = singles.tile([128, ndc, T], IODT, tag="x_sb")

        def x_dma(n, parity):
            for dc in range(parity, ndc, 2):
                eng = nc.scalar if parity else nc.sync
                eng.dma_start(
                    out=x_sb[:, dc, n * 512:(n + 1) * 512],
                    in_=xt_view[:, dc, n * 512:(n + 1) * 512])

        b_sb = [None, None, None]
        if with_qkv_bias:
            for i, bd in enumerate((bq_d, bk_d, bv_d)):
                t = singles.tile([128, npair], F32, tag=f"b{i}", name=f"b{i}")
                nc.sync.dma_start(
                    out=t[:], in_=bd.rearrange("(pr p) one -> p (pr one)",
                                               p=128))
                b_sb[i] = t
        bo_sb = None
        if with_o_bias:
            bo_sb = singles.tile([128, D], F32)
            nc.gpsimd.dma_start(out=bo_sb[:], in_=bo_d.partition_broadcast(128))

        # attention operand tiles (bf16)
        # q2tz[h]: head h's Q^T on its pair-local 64 partitions, zeros on the
        # other 64, so the [128,128] two-head K tile contracts at K=128 with
        # the other head's contribution cancelled.
        q2tz = [singles.tile([128, T], BF16, tag=f"q2tz{h}", name=f"q2tz{h}")
                for h in range(hc)]
        for h in range(hc):
            hh = h % 2
            zrows = slice(dh, 128) if hh == 0 else slice(0, dh)
            nc.vector.memset(q2tz[h][zrows, :].bitcast(F32), 0.0)
        k2t = singles.tile([128, npair, T], BF16, tag="k2t")
        out2t = singles.tile([128, npair, T], IODT, tag="out2t")
        # vnat[:, h, kt, 0:64] = V rows (k on partitions); col 64 = ones
        vnat = singles.tile([128, hc, KT, dh + 1], BF16, tag="vnat")
        onesc = singles.tile([128, hc, KT, 1], F32)
        nc.vector.memset(onesc[:], 1.0)
        nc.vector.tensor_copy(vnat[:, :, :, dh:dh + 1], onesc[:])

        # ---------- filler generators ----------
        # Each yields between ~2-matmul chunks of PE work so the drain loop
        # can weave them between score pairs without starving ACT. Only one
        # generator is open at a time (they share the 2-buf ms PSUM pool).
        def proj_gen(kind, pr, n):
            """Projection of chunk n (512 tokens) for pair pr.
            kind: 0=q, 1=k, 2=v."""
            ncol = slice(n * 512, (n + 1) * 512)
            ps = mspool.tile([128, 512], F32, tag="ms", name=f"pj{kind}{pr}{n}")
            for dc in range(ndc):
                nc.tensor.matmul(ps[:], w_sb[kind][:, dc,
                                                   pr * 128:(pr + 1) * 128],
                                 x_sb[:, dc, ncol],
                                 start=(dc == 0), stop=(dc == ndc - 1))
                if dc % 2 == 1 and dc < ndc - 1:
                    yield
            if kind == 0:
                for hh in range(2):
                    h = 2 * pr + hh
                    hp = slice(hh * dh, (hh + 1) * dh)
                    if with_qkv_bias:
                        nc.vector.tensor_scalar_add(
                            q2tz[h][hp, ncol], ps[hp, :],
                            b_sb[0][hp, pr:pr + 1])
                    else:
                        nc.vector.tensor_copy(q2tz[h][hp, ncol], ps[hp, :])
            elif kind == 1:
                if with_qkv_bias:
                    nc.vector.tensor_scalar_add(k2t[:, pr, ncol], ps[:],
                                                b_sb[1][:, pr:pr + 1])
                else:
                    nc.vector.tensor_copy(k2t[:, pr, ncol], ps[:])
            else:
                v2 = v2pool.tile([128, 512], F32R, tag="v2")
                if with_qkv_bias:
                    nc.vector.tensor_scalar_add(v2[:], ps[:],
                                                b_sb[2][:, pr:pr + 1])
                else:
                    nc.vector.tensor_copy(v2[:], ps[:])
                yield
                kt0 = n * 4
                pv = mspool.tile([128, 4, 128], F32R, tag="ms",
                                 name=f"pv{pr}{n}")
                for sub in range(4):
                    nc.tensor.transpose(pv[:, sub, :],
                                        v2[:, sub * 128:(sub + 1) * 128],
                                        identb[:])
                for hh in range(2):
                    h = 2 * pr + hh
                    nc.vector.tensor_copy(
                        vnat[:, h, kt0:kt0 + 4, 0:dh],
                        pv[:, :, hh * dh:(hh + 1) * dh])

        def fin2_gen(h, qc, po, dr):
            """Broadcast the raw denominator across dh partitions with a K=1
            matmul, fast-reciprocal it, scale the numerator into out2t."""
            pr, hh = divmod(h, 2)
            hp = slice(hh * dh, (hh + 1) * dh)
            qcol = slice(qc * 512, (qc + 1) * 512)
            den = mspool.tile([dh, 512], F32, tag="ms", name=f"den{h}_{qc}")
            nc.tensor.matmul(den[:], ones1[:], dr[:], start=True, stop=True)
            rci = rcpool.tile([dh, 512], F32, tag="rci", name=f"rci{h}_{qc}")
            nc.vector.reciprocal_approx_fast(rci[:], den[:])
            nc.vector.tensor_mul(out2t[hp, pr, qcol], po[0:dh, :], rci[:])
            return
            yield  # pragma: no cover (make this a generator)

        def p4_gen(m, j):
            """Output-projection tile: y[m*128:+128, j*NJ:+NJ]."""
            py = mspool.tile([128, NJ], F32, tag="ms", name=f"py{m}_{j}")
            nc.tensor.matmul(py[:], out2t[:, 0, m * 128:(m + 1) * 128],
                             wo_sb[:, 0, j * NJ:(j + 1) * NJ],
                             start=True, stop=(npair == 1))
            yield
            for pr in range(1, npair):
                nc.tensor.matmul(py[:], out2t[:, pr, m * 128:(m + 1) * 128],
                                 wo_sb[:, pr, j * NJ:(j + 1) * NJ],
                                 start=False, stop=(pr == npair - 1))
            yt = ysb.tile([128, NJ], BF16, tag="yt")
            if with_o_bias:
                nc.vector.tensor_add(yt[:], py[:],
                                     bo_sb[:, j * NJ:(j + 1) * NJ])
            else:
                nc.vector.tensor_copy(yt[:], py[:])
            eng = nc.sync if (m * npj + j) % 2 == 0 else nc.gpsimd
            eng.dma_start(out=y_d[m * 128:(m + 1) * 128,
                                  j * NJ:(j + 1) * NJ], in_=yt[:])

        # ---------- interleaved emission ----------
        work = deque()       # items: ("proj",k,pr,n) ("fin2",...) ("p4",m,j)
        EST = {"proj": 5, "fin2": 1, "p4": 2}
        state = {"open": None}
        done = set()         # proj items already emitted (dedup for ensure)

        def make_gen(item):
            if item[0] == "proj":
                return proj_gen(item[1], item[2], item[3])
            if item[0] == "fin2":
                return fin2_gen(item[1], item[2], item[3], item[4])
            return p4_gen(item[1], item[2])

        def pop_item():
            while work:
                item = work.popleft()
                if item[0] == "proj":
                    if item in done:
                        continue
                    done.add(item)
                return item
            return None

        def tick():
            """Advance the open generator by one segment (~2 matmuls)."""
            if state["open"] is None:
                item = pop_item()
                if item is None:
                    return False
                state["open"] = make_gen(item)
            try:
                next(state["open"])
            except StopIteration:
                state["open"] = None
            return True

        def drain_all():
            while state["open"] is not None or work:
                tick()

        def run_now(item):
            """Finish any open generator, then run item to completion."""
            while state["open"] is not None:
                tick()
            if item[0] == "proj":
                if item in done:
                    return
                done.add(item)
            state["open"] = make_gen(item)
            while state["open"] is not None:
                tick()

        def ensure_proj(kind, pr, n):
            run_now(("proj", kind, pr, n))

        # prologue DMAs: odd x slices to the scalar ring up front; sync ring
        # carries weights and even x slices in need-order, then y later
        for n in range(nch):
            x_dma(n, 1)
        for dc in range(ndc):      # K weights interleaved with x chunk 0
            nc.sync.dma_start(out=w_sb[1][:, dc, :], in_=w_views[1][:, dc, :])
            if dc % 2 == 0:
                nc.sync.dma_start(
                    out=x_sb[:, dc, 0:512], in_=xt_view[:, dc, 0:512])
        w_dma(0)                   # Q weights
        w_dma(2)                   # V weights
        for n in range(1, nch):
            x_dma(n, 0)
        nc.sync.dma_start(out=wo_sb[:],
                          in_=wo_d.rearrange("(pr p) m -> p pr m", p=128))

        # prologue projections: just enough to start (h0, qc0); later
        # chunks are pulled just-in-time by ensure_proj inside the units
        for it in (("proj", 1, 0, 0), ("proj", 0, 0, 0)):
            run_now(it)
        for n in range(nch):
            work.append(("proj", 1, 1, n))      # K p1
        work.append(("proj", 0, 1, 0))          # Q p1 chunk 0
        for n in range(nch):
            work.append(("proj", 2, 1, n))      # V p1
        work.append(("proj", 2, 0, 0))          # V p0
        for n in range(1, nch):
            work.append(("proj", 1, 0, n))      # K p0 rest
            work.append(("proj", 2, 0, n))      # V p0 rest
        for n in range(1, nch):                 # Q chunk n feeds qc == n
            work.append(("proj", 0, 0, n))
            work.append(("proj", 0, 1, n))

        units = [(h, qc) for qc in range(QC) for h in range(hc)]
        total_steps = len(units) * NK2
        pace_steps = max(1, total_steps - NK2)
        reserve = []
        step = 0
        for ui, (h, qc) in enumerate(units):
            last_unit = ui == len(units) - 1
            pr = h // 2
            qcol = slice(qc * 512, (qc + 1) * 512)
            po = popool.tile([dh + 1, 512], F32, tag="po", name=f"po{h}_{qc}")
            ensure_proj(0, pr, qc)
            prev_ex = None
            for kt2 in range(NK2):
                # just-in-time: this step's score k-tiles and po v-tiles
                ensure_proj(1, pr, (2 * kt2 + 1) // 4)
                if kt2 > 0:
                    ensure_proj(2, pr, (2 * kt2 - 2) // 4)
                sc = scpool.tile([128, 2, 512], F32, tag="sc",
                                 name=f"sc{h}_{qc}_{kt2}")
                for i in range(2):
                    kt = 2 * kt2 + i
                    kcol = slice(kt * 128, (kt + 1) * 128)
                    nc.tensor.matmul(sc[:, i, :], k2t[:, pr, kcol],
                                     q2tz[h][:, qcol], start=True, stop=True)
                ex = epool.tile([128, 2, 512], BF16, tag="ex",
                                name=f"ex{h}_{qc}_{kt2}")
                nc.scalar.activation(ex[:], sc[:], AF.Exp, scale=scale)
                if kt2 > 0:
                    for i in range(2):
                        kt = 2 * (kt2 - 1) + i
                        nc.tensor.matmul(po[:], vnat[:, h, kt, :],
                                         prev_ex[:, i, :],
                                         start=(kt == 0), stop=False)
                prev_ex = ex
                # weave filler segments, paced to drain before the last unit
                if last_unit and reserve:
                    run_now(reserve.pop(0))
                est = sum(EST[it[0]] for it in work)
                steps_left = max(1, pace_steps - step)
                nticks = min(3, max(0, (est + steps_left - 1) // steps_left)
                             if (work or state["open"]) else 0)
                if state["open"] is not None and nticks == 0:
                    nticks = 1
                for _ in range(nticks):
                    tick()
                step += 1
            for i in range(2):
                kt = 2 * (NK2 - 1) + i
                nc.tensor.matmul(po[:], vnat[:, h, kt, :], prev_ex[:, i, :],
                                 start=(kt == 0), stop=(kt == KT - 1))
            # raw denominator to SBUF now (f32r for the broadcast matmul);
            # the broadcast+reciprocal+scale chain is queued so its latency
            # hides behind the next unit's first score matmuls
            dr = rcpool.tile([1, 512], F32R, tag="dr", name=f"dr{h}_{qc}")
            nc.vector.tensor_copy(dr[:], po[dh:dh + 1, :])
            work.appendleft(("fin2", h, qc, po, dr))
            if h == hc - 1:
                tiles = [("p4", m, j)
                         for m in range(qc * 512 // 128, (qc + 1) * 512 // 128)
                         for j in range(npj)]
                if QC > 1 and qc == QC - 2:
                    keep = min(len(tiles), NK2 // 2 + 2)
                    reserve = tiles[-keep:]
                    tiles = tiles[:-keep]
                work.extend(tiles)
        # tail drain: run the final fin2 first, then the reserved tiles
        # (independent PE work) to hide the normalize chain, then the rest
        if work and work[0][0] == "fin2":
            run_now(work.popleft())
        work.extendleft(reversed(reserve))
        drain_all()

        if debug_taps:
            dq = nc.dram_tensor("dbg_q", [128, hc, T], F32,
                                kind="ExternalOutput").ap()
            dk = nc.dram_tensor("dbg_k", [128, npair, T], F32,
                                kind="ExternalOutput").ap()
            dv = nc.dram_tensor("dbg_v", [128, hc, KT, dh + 1], F32,
                                kind="ExternalOutput").ap()
            do = nc.dram_tensor("dbg_o", [128, npair, T], F32,
                                kind="ExternalOutput").ap()
            with tc.tile_pool(name="dbg", bufs=1) as dbg:
                tq = dbg.tile([128, hc, T], F32)
                for h in range(hc):
                    nc.vector.tensor_copy(tq[:, h, :], q2tz[h][:])
                nc.sync.dma_start(out=dq, in_=tq[:])
                tk = dbg.tile([128, npair, T], F32)
                nc.vector.tensor_copy(tk[:], k2t[:])
                nc.sync.dma_start(out=dk, in_=tk[:])
                tv = dbg.tile([128, hc, KT, dh + 1], F32)
                nc.vector.tensor_copy(tv[:], vnat[:])
                nc.sync.dma_start(out=dv, in_=tv[:])
                to = dbg.tile([128, npair, T], F32)
                nc.vector.tensor_copy(to[:], out2t[:])
                nc.sync.dma_start(out=do, in_=to[:])

    nc.compile()
    return nc


_PROGRAM_CACHE = {}


def _get_program(key):
    if key not in _PROGRAM_CACHE:
        with_qkv_bias, with_o_bias = key
        _PROGRAM_CACHE[key] = build_program(
            with_qkv_bias=with_qkv_bias, with_o_bias=with_o_bias,
            bf16_io=BF16_IO)
    return _PROGRAM_CACHE[key]


def _round_tf32(a):
    """Round fp32 to tf32 (10-bit mantissa), round-to-nearest-even."""
    u = np.ascontiguousarray(a).view(np.uint32)
    r = (u + 0xFFF + ((u >> 13) & 1)) & np.uint32(0xFFFFE000)
    return r.view(np.float32)


BF16_IO = True


def _to_bf16(a):
    """fp32 -> bf16 (RTNE), or tf32-rounded fp32 when BF16_IO is off."""
    if BF16_IO:
        return np.ascontiguousarray(a).astype(ml_dtypes.bfloat16)
    return _round_tf32(np.ascontiguousarray(a, np.float32))


def make_in_maps(x, wq, bq, wk, bk, wv, bv, wo, bo, with_qkv_bias, with_o_bias,
                 S=S_FULL, hc=HEADS_PER_CORE, dh=DEPTH):
    """x: [B*S, D] fp32. Core c gets batch c // CORES_PER_BATCH and head
    columns d2*(c % CORES_PER_BATCH) .. +d2."""
    d2 = hc * dh
    nb = x.shape[0] // S
    xts = [_to_bf16(x[b * S:(b + 1) * S].T) for b in range(nb)]
    wqh, wkh, wvh, woh = (_to_bf16(w) for w in (wq, wk, wv, wo))
    in_maps = []
    for c in range(N_CORES):
        g = c % CORES_PER_BATCH
        cs = slice(g * d2, (g + 1) * d2)
        m = {"xt": xts[c // CORES_PER_BATCH],
             "wq": np.ascontiguousarray(wqh[:, cs]),
             "wk": np.ascontiguousarray(wkh[:, cs]),
             "wv": np.ascontiguousarray(wvh[:, cs]),
             "wo": np.ascontiguousarray(woh[cs, :])}
        if with_qkv_bias:
            m["bq"] = np.ascontiguousarray(bq[cs].reshape(d2, 1))
            m["bk"] = np.ascontiguousarray(bk[cs].reshape(d2, 1))
            m["bv"] = np.ascontiguousarray(bv[cs].reshape(d2, 1))
        if with_o_bias:
            m["bo"] = (bo.reshape(1, -1).astype(np.float32) if g == 0
                       else np.zeros((1, bo.shape[-1]), np.float32))
        in_maps.append(m)
    return in_maps


def kernel(inputs, wq, bq, wk, bk, wv, bv, wo, bo):
    x = np.ascontiguousarray(np.asarray(inputs, np.float32)
                             .reshape(B_FULL * S_FULL, D_MODEL))
    wq, wk, wv, wo = (np.asarray(a, np.float32) for a in (wq, wk, wv, wo))
    bq, bk, bv, bo = (np.asarray(a, np.float32) for a in (bq, bk, bv, bo))

    with_qkv_bias = bool(np.any(bq) or np.any(bk) or np.any(bv))
    with_o_bias = bool(np.any(bo))
    nc = _get_program((with_qkv_bias, with_o_bias))

    in_maps = make_in_maps(x, wq, bq, wk, bk, wv, bv, wo, bo,
                           with_qkv_bias, with_o_bias)
    res = run_bass_kernel_spmd(nc, in_maps, list(range(N_CORES))).results
    y = np.zeros((B_FULL, S_FULL, D_MODEL), np.float64)
    for c in range(N_CORES):
        y[c // CORES_PER_BATCH] += res[c]["y"]
    return y.astype(np.float32)


# revision 25
# speedup vs baseline: 1.0765x; 1.0025x over previous
"""Multi-head self-attention (B=2, S=2048, D=1024, H=16) on 8 TRN2 NeuronCores.

Sharding: batch x head-group parallel. Core c handles batch c//4 and the
4 heads 4*(c%4)..4*(c%4)+3. Each core reads only its batch's tokens,
computes Q/K/V for its 4 heads, full non-causal softmax attention for its
4 (head) units, and the partial output projection y_c = sum_h out_h@wo[h].
Host sums the 4 partials per batch. Host pre-transposes x to xT and
pre-rounds x and all weights to bf16 so the head of the kernel (which is
HBM-bound on the x + weight loads) moves half the bytes.

Device dataflow per core (heads grouped in 2 pairs of 2; head-dim on
partitions; pair p owns partitions [0:64] for head 2p, [64:128] for 2p+1):
  k2t[p] / q2tz[h] / out2t[p] [128, T]; x resident in SBUF [128, ndc, T]
  proj: psum = sum_dc w[:,dc,p*128:+128]^T @ x[:,dc,chunk]   (PSUM accum)
  v --PE transpose--> vnat[h] [k, 64] bf16 (+ ones column -> denominator)
  scoresT[k, q] = K Q^T with per-head zero-padded Q (K=128 contraction);
    score matmuls write PAIRS of PSUM banks, one ACT exp instruction
    covers both ([128, 1024]) to amortize ACT fixed overheads
  poT[65, q] += vnat[h,kt]^T @ exp  accumulated over kt (PSUM)
  normalize: dr=[1,512] raw denominator (f32r) -> K=1 matmul broadcast ->
    reciprocal_approx_fast -> out2t[hp, q] = po[0:64] * rcp (DVE)
  y[s, :] = sum_p out2t[p][:, s-tile]^T @ wo[p]  -> SBUF -> DMA

All matmul operands are bf16 except the f32r denominator broadcast and
the f32r V-transpose path; PSUM accumulation is fp32. bf16 and f32r both
stream 1 PE row/cycle, so this costs no PE time, only accuracy that stays
well inside the 2e-2 budget.

Scheduling: q-chunk-major (h, qc) units; exp runs on pairs of k-tiles.
During attention the ACT engine is the local pacer, so all filler work
(remaining projections, output-projection tiles, normalizations) is
emitted through generators that yield every ~2 matmuls, weaving at most a
few hundred ns of PE work between consecutive score pairs — a coarse
filler burst would starve ACT. x DMAs split across the sync and scalar
hwdge rings (the two rings share HBM but halve serialization); y writes
alternate sync/gpsimd rings so the tail is not bound by one ring.
"""

import numpy as np
from collections import deque
from contextlib import ExitStack

import ml_dtypes
import concourse.bass as bass
import concourse.tile as tile
from concourse import bacc, mybir
from concourse.bass_utils import run_bass_kernel_spmd
from concourse.masks import make_identity

F32 = mybir.dt.float32
F32R = mybir.dt.float32r
BF16 = mybir.dt.bfloat16
AF = mybir.ActivationFunctionType

N_CORES = 8
D_MODEL = 1024
NUM_HEADS = 16
DEPTH = 64
B_FULL = 2
S_FULL = 2048
CORES_PER_BATCH = N_CORES // B_FULL          # 4
HEADS_PER_CORE = NUM_HEADS // CORES_PER_BATCH  # 4


def build_program(T=2048, D=1024, dh=64, hc=4, with_qkv_bias=False,
                  with_o_bias=False, debug_taps=False, bf16_io=True):
    """SPMD Bass program for one core: hc heads over T tokens (one batch).

    Requires hc == 4, dh == 64, D % 128 == 0, T % 512 == 0.
    """
    d2 = hc * dh                 # 256
    npair = d2 // 128            # 2
    assert npair == 2 and dh == 64 and D % 128 == 0 and T % 512 == 0
    S = T                        # single batch per core
    ndc = D // 128               # contraction chunks
    nch = T // 512               # 512-token chunks
    KT = S // 128                # k tiles per unit
    NK2 = KT // 2                # score-pair steps per unit
    QC = S // 512                # q chunks
    NJ = min(512, D)
    npj = D // NJ                # output-proj j tiles per m tile
    scale = 1.0 / float(np.sqrt(dh))

    nc = bacc.Bacc("TRN2", target_bir_lowering=False, debug=False,
                   num_devices=N_CORES)
    IODT = BF16 if bf16_io else F32R

    xt_d = nc.dram_tensor("xt", [D, T], IODT, kind="ExternalInput").ap()
    wq_d = nc.dram_tensor("wq", [D, d2], IODT, kind="ExternalInput").ap()
    wk_d = nc.dram_tensor("wk", [D, d2], IODT, kind="ExternalInput").ap()
    wv_d = nc.dram_tensor("wv", [D, d2], IODT, kind="ExternalInput").ap()
    wo_d = nc.dram_tensor("wo", [d2, D], IODT, kind="ExternalInput").ap()
    if with_qkv_bias:
        bq_d = nc.dram_tensor("bq", [d2, 1], F32, kind="ExternalInput").ap()
        bk_d = nc.dram_tensor("bk", [d2, 1], F32, kind="ExternalInput").ap()
        bv_d = nc.dram_tensor("bv", [d2, 1], F32, kind="ExternalInput").ap()
    if with_o_bias:
        bo_d = nc.dram_tensor("bo", [1, D], F32, kind="ExternalInput").ap()
    y_d = nc.dram_tensor("y", [T, D], BF16, kind="ExternalOutput").ap()

    xt_view = xt_d.rearrange("(dc p) t -> p dc t", p=128)

    with tile.TileContext(nc) as tc, ExitStack() as ctx:
        singles = ctx.enter_context(tc.tile_pool(name="singles", bufs=1))
        v2pool = ctx.enter_context(tc.tile_pool(name="v2pool", bufs=2))
        epool = ctx.enter_context(tc.tile_pool(name="epool", bufs=3))
        ysb = ctx.enter_context(tc.tile_pool(name="ysb", bufs=4))
        rcpool = ctx.enter_context(tc.tile_pool(name="rcpool", bufs=4))
        # PSUM budget (8 banks): sc 2x[128,2,512]=4, po 2x[65,512]=2,
        # ms 2x[128,512]=2
        scpool = ctx.enter_context(tc.tile_pool(name="sc", bufs=2,
                                                space="PSUM"))
        popool = ctx.enter_context(tc.tile_pool(name="po", bufs=2,
                                                space="PSUM"))
        mspool = ctx.enter_context(tc.tile_pool(name="ms", bufs=2,
                                                space="PSUM"))

        identf = singles.tile([128, 128], F32)
        make_identity(nc, identf[:])
        # touch Exp once so the ACT table load happens during the DMA wait
        warm = singles.tile([1, 2], F32)
        nc.vector.memset(warm[:], 0.0)
        warmo = singles.tile([1, 2], F32)
        nc.scalar.activation(warmo[:], warm[:], AF.Exp)
        identb = singles.tile([128, 128], F32R)
        nc.vector.tensor_copy(identb[:], identf[:])
        ones1f = singles.tile([1, dh], F32)
        nc.vector.memset(ones1f[:], 1.0)
        ones1 = singles.tile([1, dh], F32R)
        nc.vector.tensor_copy(ones1[:], ones1f[:])

        # Weight DMAs split per-dc, interleaved with x loads in need-order
        w_sb = [singles.tile([128, ndc, d2], IODT, tag=n, name=n)
                for n in ("wqs", "wks", "wvs")]
        w_views = [wd.rearrange("(dc p) m -> p dc m", p=128)
                   for wd in (wq_d, wk_d, wv_d)]
        wo_sb = singles.tile([128, npair, D], IODT)

        def w_dma(kind, eng):
            for dc in range(ndc):
                eng.dma_start(out=w_sb[kind][:, dc, :],
                              in_=w_views[kind][:, dc, :])

        # resident x; odd dc slices ride the scalar (Activation) hwdge ring,
        # even slices the sync ring
        x_sb = singles.tile([128, ndc, T], IODT, tag="x_sb")

        def x_dma(n, parity):
            for dc in range(parity, ndc, 2):
                eng = nc.gpsimd if parity else nc.sync
                eng.dma_start(
                    out=x_sb[:, dc, n * 512:(n + 1) * 512],
                    in_=xt_view[:, dc, n * 512:(n + 1) * 512])

        b_sb = [None, None, None]
        if with_qkv_bias:
            for i, bd in enumerate((bq_d, bk_d, bv_d)):
                t = singles.tile([128, npair], F32, tag=f"b{i}", name=f"b{i}")
                nc.sync.dma_start(
                    out=t[:], in_=bd.rearrange("(pr p) one -> p (pr one)",
                                               p=128))
                b_sb[i] = t
        bo_sb = None
        if with_o_bias:
            bo_sb = singles.tile([128, D], F32)
            nc.gpsimd.dma_start(out=bo_sb[:], in_=bo_d.partition_broadcast(128))

        # attention operand tiles (bf16)
        # q2tz[h]: head h's Q^T on its pair-local 64 partitions, zeros on the
        # other 64, so the [128,128] two-head K tile contracts at K=128 with
        # the other head's contribution cancelled.
        q2tz = [singles.tile([128, T], BF16, tag=f"q2tz{h}", name=f"q2tz{h}")
                for h in range(hc)]
        for h in range(hc):
            hh = h % 2
            zrows = slice(dh, 128) if hh == 0 else slice(0, dh)
            nc.vector.memset(q2tz[h][zrows, :].bitcast(F32), 0.0)
        k2t = singles.tile([128, npair, T], BF16, tag="k2t")
        out2t = singles.tile([128, npair, T], IODT, tag="out2t")
        # vnat[:, h, kt, 0:64] = V rows (k on partitions); col 64 = ones
        vnat = singles.tile([128, hc, KT, dh + 1], BF16, tag="vnat")
        onesc = singles.tile([128, hc, KT, 1], F32)
        nc.vector.memset(onesc[:], 1.0)
        nc.vector.tensor_copy(vnat[:, :, :, dh:dh + 1], onesc[:])

        # ---------- filler generators ----------
        # Each yields between ~2-matmul chunks of PE work so the drain loop
        # can weave them between score pairs without starving ACT. Only one
        # generator is open at a time (they share the 2-buf ms PSUM pool).
        def proj_gen(kind, pr, n):
            """Projection of chunk n (512 tokens) for pair pr.
            kind: 0=q, 1=k, 2=v."""
            ncol = slice(n * 512, (n + 1) * 512)
            ps = mspool.tile([128, 512], F32, tag="ms", name=f"pj{kind}{pr}{n}")
            for dc in range(ndc):
                nc.tensor.matmul(ps[:], w_sb[kind][:, dc,
                                                   pr * 128:(pr + 1) * 128],
                                 x_sb[:, dc, ncol],
                                 start=(dc == 0), stop=(dc == ndc - 1))
                if dc % 2 == 1 and dc < ndc - 1:
                    yield
            if kind == 0:
                for hh in range(2):
                    h = 2 * pr + hh
                    hp = slice(hh * dh, (hh + 1) * dh)
                    if with_qkv_bias:
                        nc.vector.tensor_scalar_add(
                            q2tz[h][hp, ncol], ps[hp, :],
                            b_sb[0][hp, pr:pr + 1])
                    else:
                        nc.vector.tensor_copy(q2tz[h][hp, ncol], ps[hp, :])
            elif kind == 1:
                if with_qkv_bias:
                    nc.vector.tensor_scalar_add(k2t[:, pr, ncol], ps[:],
                                                b_sb[1][:, pr:pr + 1])
                else:
                    nc.vector.tensor_copy(k2t[:, pr, ncol], ps[:])
            else:
                v2 = v2pool.tile([128, 512], F32R, tag="v2")
                if with_qkv_bias:
                    nc.vector.tensor_scalar_add(v2[:], ps[:],
                                                b_sb[2][:, pr:pr + 1])
                else:
                    nc.vector.tensor_copy(v2[:], ps[:])
                yield
                kt0 = n * 4
                pv = mspool.tile([128, 4, 128], F32R, tag="ms",
                                 name=f"pv{pr}{n}")
                for sub in range(4):
                    nc.tensor.transpose(pv[:, sub, :],
                                        v2[:, sub * 128:(sub + 1) * 128],
                                        identb[:])
                for hh in range(2):
                    h = 2 * pr + hh
                    nc.vector.tensor_copy(
                        vnat[:, h, kt0:kt0 + 4, 0:dh],
                        pv[:, :, hh * dh:(hh + 1) * dh])

        def fin2_gen(h, qc, po, dr):
            """Broadcast the raw denominator across dh partitions with a K=1
            matmul, fast-reciprocal it, scale the numerator into out2t."""
            pr, hh = divmod(h, 2)
            hp = slice(hh * dh, (hh + 1) * dh)
            qcol = slice(qc * 512, (qc + 1) * 512)
            den = mspool.tile([dh, 512], F32, tag="ms", name=f"den{h}_{qc}")
            nc.tensor.matmul(den[:], ones1[:], dr[:], start=True, stop=True)
            rci = rcpool.tile([dh, 512], F32, tag="rci", name=f"rci{h}_{qc}")
            nc.vector.reciprocal_approx_fast(rci[:], den[:])
            nc.vector.tensor_mul(out2t[hp, pr, qcol], po[0:dh, :], rci[:])
            return
            yield  # pragma: no cover (make this a generator)

        def p4_gen(m, j):
            """Output-projection tile: y[m*128:+128, j*NJ:+NJ]."""
            py = mspool.tile([128, NJ], F32, tag="ms", name=f"py{m}_{j}")
            nc.tensor.matmul(py[:], out2t[:, 0, m * 128:(m + 1) * 128],
                             wo_sb[:, 0, j * NJ:(j + 1) * NJ],
                             start=True, stop=(npair == 1))
            yield
            for pr in range(1, npair):
                nc.tensor.matmul(py[:], out2t[:, pr, m * 128:(m + 1) * 128],
                                 wo_sb[:, pr, j * NJ:(j + 1) * NJ],
                                 start=False, stop=(pr == npair - 1))
            yt = ysb.tile([128, NJ], BF16, tag="yt")
            if with_o_bias:
                nc.vector.tensor_add(yt[:], py[:],
                                     bo_sb[:, j * NJ:(j + 1) * NJ])
            else:
                nc.vector.tensor_copy(yt[:], py[:])
            if (m * npj + j) % 2 == 0:
                eng = nc.sync
            elif m >= (QC - 1) * 512 // 128:
                eng = nc.scalar
            else:
                eng = nc.gpsimd
            eng.dma_start(out=y_d[m * 128:(m + 1) * 128,
                                  j * NJ:(j + 1) * NJ], in_=yt[:])

        # ---------- interleaved emission ----------
        work = deque()       # items: ("proj",k,pr,n) ("fin2",...) ("p4",m,j)
        EST = {"proj": 5, "fin2": 1, "p4": 2}
        state = {"open": None}
        done = set()         # proj items already emitted (dedup for ensure)

        def make_gen(item):
            if item[0] == "proj":
                return proj_gen(item[1], item[2], item[3])
            if item[0] == "fin2":
                return fin2_gen(item[1], item[2], item[3], item[4])
            return p4_gen(item[1], item[2])

        def pop_item():
            while work:
                item = work.popleft()
                if item[0] == "proj":
                    if item in done:
                        continue
                    done.add(item)
                return item
            return None

        def tick():
            """Advance the open generator by one segment (~2 matmuls)."""
            if state["open"] is None:
                item = pop_item()
                if item is None:
                    return False
                state["open"] = make_gen(item)
            try:
                next(state["open"])
            except StopIteration:
                state["open"] = None
            return True

        def drain_all():
            while state["open"] is not None or work:
                tick()

        def run_now(item):
            """Finish any open generator, then run item to completion."""
            while state["open"] is not None:
                tick()
            if item[0] == "proj":
                if item in done:
                    return
                done.add(item)
            state["open"] = make_gen(item)
            while state["open"] is not None:
                tick()

        def ensure_proj(kind, pr, n):
            run_now(("proj", kind, pr, n))

        # prologue DMAs: odd x slices to the scalar ring up front; sync ring
        # carries weights and even x slices in need-order, then y later
        for n in range(nch):
            x_dma(n, 1)
        for dc in range(ndc):      # K weights interleaved with x chunk 0
            nc.sync.dma_start(out=w_sb[1][:, dc, :], in_=w_views[1][:, dc, :])
            if dc % 2 == 0:
                nc.sync.dma_start(
                    out=x_sb[:, dc, 0:512], in_=xt_view[:, dc, 0:512])
        w_dma(0, nc.scalar)        # Q weights on the scalar ring
        w_dma(2, nc.scalar)        # V weights
        for n in range(1, nch):
            x_dma(n, 0)
        nc.sync.dma_start(out=wo_sb[:],
                          in_=wo_d.rearrange("(pr p) m -> p pr m", p=128))

        # prologue projections: just enough to start (h0, qc0); later
        # chunks are pulled just-in-time by ensure_proj inside the units
        for it in (("proj", 1, 0, 0), ("proj", 0, 0, 0)):
            run_now(it)
        for n in range(nch):
            work.append(("proj", 1, 1, n))      # K p1
        work.append(("proj", 0, 1, 0))          # Q p1 chunk 0
        for n in range(nch):
            work.append(("proj", 2, 1, n))      # V p1
        work.append(("proj", 2, 0, 0))          # V p0
        for n in range(1, nch):
            work.append(("proj", 1, 0, n))      # K p0 rest
            work.append(("proj", 2, 0, n))      # V p0 rest
        for n in range(1, nch):                 # Q chunk n feeds qc == n
            work.append(("proj", 0, 0, n))
            work.append(("proj", 0, 1, n))

        units = [(h, qc) for qc in range(QC) for h in range(hc)]
        total_steps = len(units) * NK2
        pace_steps = max(1, total_steps - NK2)
        reserve = []
        step = 0
        for ui, (h, qc) in enumerate(units):
            last_unit = ui == len(units) - 1
            pr = h // 2
            qcol = slice(qc * 512, (qc + 1) * 512)
            po = popool.tile([dh + 1, 512], F32, tag="po", name=f"po{h}_{qc}")
            ensure_proj(0, pr, qc)
            prev_ex = None
            for kt2 in range(NK2):
                # just-in-time: this step's score k-tiles and po v-tiles
                ensure_proj(1, pr, (2 * kt2 + 1) // 4)
                if kt2 > 0:
                    ensure_proj(2, pr, (2 * kt2 - 2) // 4)
                sc = scpool.tile([128, 2, 512], F32, tag="sc",
                                 name=f"sc{h}_{qc}_{kt2}")
                for i in range(2):
                    kt = 2 * kt2 + i
                    kcol = slice(kt * 128, (kt + 1) * 128)
                    nc.tensor.matmul(sc[:, i, :], k2t[:, pr, kcol],
                                     q2tz[h][:, qcol], start=True, stop=True)
                ex = epool.tile([128, 2, 512], BF16, tag="ex",
                                name=f"ex{h}_{qc}_{kt2}")
                nc.scalar.activation(ex[:], sc[:], AF.Exp, scale=scale)
                if kt2 > 0:
                    for i in range(2):
                        kt = 2 * (kt2 - 1) + i
                        nc.tensor.matmul(po[:], vnat[:, h, kt, :],
                                         prev_ex[:, i, :],
                                         start=(kt == 0), stop=False)
                prev_ex = ex
                # weave filler segments, paced to drain before the last unit
                if last_unit and reserve:
                    run_now(reserve.pop(0))
                est = sum(EST[it[0]] for it in work)
                steps_left = max(1, pace_steps - step)
                nticks = min(3, max(0, (est + steps_left - 1) // steps_left)
                             if (work or state["open"]) else 0)
                if state["open"] is not None and nticks == 0:
                    nticks = 1
                for _ in range(nticks):
                    tick()
                step += 1
            for i in range(2):
                kt = 2 * (NK2 - 1) + i
                nc.tensor.matmul(po[:], vnat[:, h, kt, :], prev_ex[:, i, :],
                                 start=(kt == 0), stop=(kt == KT - 1))
            # raw denominator to SBUF now (f32r for the broadcast matmul);
            # the broadcast+reciprocal+scale chain is queued so its latency
            # hides behind the next unit's first score matmuls
            dr = rcpool.tile([1, 512], F32R, tag="dr", name=f"dr{h}_{qc}")
            nc.vector.tensor_copy(dr[:], po[dh:dh + 1, :])
            work.appendleft(("fin2", h, qc, po, dr))
            if h == hc - 1:
                tiles = [("p4", m, j)
                         for m in range(qc * 512 // 128, (qc + 1) * 512 // 128)
                         for j in range(npj)]
                if QC > 1 and qc == QC - 2:
                    keep = min(len(tiles), NK2 // 2 + 2)
                    reserve = tiles[-keep:]
                    tiles = tiles[:-keep]
                work.extend(tiles)
        # tail drain: run the final fin2 first, then the reserved tiles
        # (independent PE work) to hide the normalize chain, then the rest
        if work and work[0][0] == "fin2":
            run_now(work.popleft())
        work.extendleft(reversed(reserve))
        drain_all()

        if debug_taps:
            dq = nc.dram_tensor("dbg_q", [128, hc, T], F32,
                                kind="ExternalOutput").ap()
            dk = nc.dram_tensor("dbg_k", [128, npair, T], F32,
                                kind="ExternalOutput").ap()
            dv = nc.dram_tensor("dbg_v", [128, hc, KT, dh + 1], F32,
                                kind="ExternalOutput").ap()
            do = nc.dram_tensor("dbg_o", [128, npair, T], F32,
                                kind="ExternalOutput").ap()
            with tc.tile_pool(name="dbg", bufs=1) as dbg:
                tq = dbg.tile([128, hc, T], F32)
                for h in range(hc):
                    nc.vector.tensor_copy(tq[:, h, :], q2tz[h][:])
                nc.sync.dma_start(out=dq, in_=tq[:])
                tk = dbg.tile([128, npair, T], F32)
                nc.vector.tensor_copy(tk[:], k2t[:])
                nc.sync.dma_start(out=dk, in_=tk[:])
                tv = dbg.tile([128, hc, KT, dh + 1], F32)
                nc.vector.tensor_copy(tv[:], vnat[:])
                nc.sync.dma_start(out=dv, in_=tv[:])
                to = dbg.tile([128, npair, T], F32)
                nc.vector.tensor_copy(to[:], out2t[:])
                nc.sync.dma_start(out=do, in_=to[:])

    nc.compile()
    return nc


_PROGRAM_CACHE = {}


def _get_program(key):
    if key not in _PROGRAM_CACHE:
        with_qkv_bias, with_o_bias = key
        _PROGRAM_CACHE[key] = build_program(
            with_qkv_bias=with_qkv_bias, with_o_bias=with_o_bias,
            bf16_io=BF16_IO)
    return _PROGRAM_CACHE[key]


def _round_tf32(a):
    """Round fp32 to tf32 (10-bit mantissa), round-to-nearest-even."""
    u = np.ascontiguousarray(a).view(np.uint32)
    r = (u + 0xFFF + ((u >> 13) & 1)) & np.uint32(0xFFFFE000)
    return r.view(np.float32)


BF16_IO = True


def _to_bf16(a):
    """fp32 -> bf16 (RTNE), or tf32-rounded fp32 when BF16_IO is off."""
    if BF16_IO:
        return np.ascontiguousarray(a).astype(ml_dtypes.bfloat16)
    return _round_tf32(np.ascontiguousarray(a, np.float32))


def make_in_maps(x, wq, bq, wk, bk, wv, bv, wo, bo, with_qkv_bias, with_o_bias,
                 S=S_FULL, hc=HEADS_PER_CORE, dh=DEPTH):
    """x: [B*S, D] fp32. Core c gets batch c // CORES_PER_BATCH and head
    columns d2*(c % CORES_PER_BATCH) .. +d2."""
    d2 = hc * dh
    nb = x.shape[0] // S
    xts = [_to_bf16(x[b * S:(b + 1) * S].T) for b in range(nb)]
    wqh, wkh, wvh, woh = (_to_bf16(w) for w in (wq, wk, wv, wo))
    in_maps = []
    for c in range(N_CORES):
        g = c % CORES_PER_BATCH
        cs = slice(g * d2, (g + 1) * d2)
        m = {"xt": xts[c // CORES_PER_BATCH],
             "wq": np.ascontiguousarray(wqh[:, cs]),
             "wk": np.ascontiguousarray(wkh[:, cs]),
             "wv": np.ascontiguousarray(wvh[:, cs]),
             "wo": np.ascontiguousarray(woh[cs, :])}
        if with_qkv_bias:
            m["bq"] = np.ascontiguousarray(bq[cs].reshape(d2, 1))
            m["bk"] = np.ascontiguousarray(bk[cs].reshape(d2, 1))
            m["bv"] = np.ascontiguousarray(bv[cs].reshape(d2, 1))
        if with_o_bias:
            m["bo"] = (bo.reshape(1, -1).astype(np.float32) if g == 0
                       else np.zeros((1, bo.shape[-1]), np.float32))
        in_maps.append(m)
    return in_maps


def kernel(inputs, wq, bq, wk, bk, wv, bv, wo, bo):
    x = np.ascontiguousarray(np.asarray(inputs, np.float32)
                             .reshape(B_FULL * S_FULL, D_MODEL))
    wq, wk, wv, wo = (np.asarray(a, np.float32) for a in (wq, wk, wv, wo))
    bq, bk, bv, bo = (np.asarray(a, np.float32) for a in (bq, bk, bv, bo))

    with_qkv_bias = bool(np.any(bq) or np.any(bk) or np.any(bv))
    with_o_bias = bool(np.any(bo))
    nc = _get_program((with_qkv_bias, with_o_bias))

    in_maps = make_in_maps(x, wq, bq, wk, bk, wv, bv, wo, bo,
                           with_qkv_bias, with_o_bias)
    res = run_bass_kernel_spmd(nc, in_maps, list(range(N_CORES))).results
    y = np.zeros((B_FULL, S_FULL, D_MODEL), np.float64)
    for c in range(N_CORES):
        y[c // CORES_PER_BATCH] += res[c]["y"]
    return y.astype(np.float32)
